# revision 12
# baseline (speedup 1.0000x reference)
"""EquivariantGNN layer on 8 Trainium2 NeuronCores.

Data-parallel over the 256 graphs (32 graphs/core, processed as 16
pairs packed on 128 partitions). Per pair, the N^2 edge work lives in a
[128, 4096] "transposed" layout: partitions = (graph, channel), free =
(j, i) with i innermost.

Algebraic restructuring vs the reference:
  - edge MLP layer 1 is separable: e_in @ W1 = HA[i] + HB[j] + w1r*d2
    -> built by PE matmuls straight into PSUM (identity-pattern rhs),
    b1 folded into the relu bias.
  - coord MLP layer 1 is fused into edge layer 2: t = r1 @ (W2@C1),
    so m_ij is never materialized; m_i = (sum_j r1) @ W2.
  - pos aggregation via sum_j w_ij * [pos_j|1] = one K=128 matmul after
    un-flattening w into a block-diagonal [128,128] tile by DMA.
"""

import os
import sys

sys.path.insert(0, "/opt/trn_rl_repo")

# Recovery knob for a previously wedged NeuronCore (NRT_EXEC_UNIT_UNRECOVERABLE):
# only read at runtime init, never overrides an explicit setting.
os.environ.setdefault("NEURON_RT_RESET_CORES", "1")

import numpy as np

import concourse.bass as bass
import concourse.tile as tile
from concourse import bacc, mybir
from concourse.bass_utils import run_bass_kernel_spmd

N = 64          # nodes per graph
H = 64          # hidden
D = 6           # in/out feature dim
B, S = 8, 32
G = B * S       # 256 graphs
NCORES = 8
GPC = G // NCORES   # 32 graphs per core
PAIRS = GPC // 2    # 16 pairs
NSQ = N * N         # 4096
CH = 512            # free-dim chunk (one PSUM bank of f32)
NCH = NSQ // CH     # 8 chunks per pair
JB = CH // N        # 8 j-values per chunk

F32 = mybir.dt.float32
F32R = mybir.dt.float32r
BF16 = mybir.dt.bfloat16

# ---- perf knobs ----
MM_F32R = False     # bitcast f32 matmul operands to float32r (4x PE rate)
R1_BF16 = False     # r1/u intermediates in bf16 (faster DVE, ~4e-3 rounding)

R1DT = BF16 if R1_BF16 else F32
AF = mybir.ActivationFunctionType
OP = mybir.AluOpType


def _mm(ap):
    """Matmul-operand view: float32 -> float32r when enabled."""
    if MM_F32R and ap.dtype == F32:
        return ap.bitcast(F32R)
    return ap


def _mmdt(dt):
    if MM_F32R and dt == F32:
        return F32R
    return dt


def build_program():
    nc = bacc.Bacc("TRN2", target_bir_lowering=False, debug=False)
    dt_r1 = R1DT

    # ---------------- DRAM I/O ----------------
    xD = nc.dram_tensor("x", [GPC, N, D], F32, kind="ExternalInput").ap()
    outD = nc.dram_tensor("out", [GPC, N, D], F32, kind="ExternalOutput").ap()

    def cin(name, shape, dt=F32):
        return nc.dram_tensor(name, list(shape), dt, kind="ExternalInput").ap()

    embbdD = cin("embbd", [38, 2 * H])              # padded blockdiag emb_w
    I128D = cin("I128", [2 * H, 2 * H])             # identity for transpose
    b1nrD = cin("b1nr", [1, 2 * H])                 # node_b1 dup as row
    fb3rD = cin("fb3r", [1, 3])                     # final_b[3:] row
    onesrD = cin("onesr", [1, 2 * H])               # ones row
    embbD = cin("embb", [2 * H, 1])                 # emb_b dup
    AbdD = cin("Abd", [2 * H, 2 * H])               # blockdiag edge_w1[:H]
    BbdD = cin("Bbd", [2 * H, 2 * H])               # blockdiag edge_w1[H:2H]
    w1rmD = cin("w1rm", [2, 2 * H])                 # masked w1r rows
    b1cD = cin("b1c", [2 * H, 1])                   # edge_b1 dup
    TbigD = cin("Tbig", [N, NSQ])                   # delta_{i,i'} pattern
    Tbig2D = cin("Tbig2", [N, NSQ])                 # delta_{j,j'} pattern
    maskbdD = cin("maskbd", [2, 2 * H])             # graph masks
    W2C1bdD = cin("W2C1bd", [2 * H, 2 * H], dt_r1)  # blockdiag edge_w2@coord_w1
    b2c1cD = cin("b2c1c", [2 * H, 1])               # (b2@C1 + c1b) dup
    c2scD = cin("c2sc", [2 * H, 16 * NCH], dt_r1)   # shifted coord_w2 cols
    W2bdD = cin("W2bd", [2 * H, 2 * H], dt_r1)      # blockdiag edge_w2
    b2x64D = cin("b2x64", [2 * H, 1])               # 64*edge_b2 dup
    W1nh2D = cin("W1nh2", [2 * H, H])               # node_w1[:H] dup'd
    W1nm2D = cin("W1nm2", [2 * H, H])               # node_w1[H:] dup'd
    W2nbdD = cin("W2nbd", [2 * H, 2 * H])           # blockdiag node_w2
    b2ncD = cin("b2nc", [2 * H, 1])                 # node_b2 dup
    Wf3D = cin("Wf3", [2 * H, 3])                   # final_w[:, 3:] dup'd

    from contextlib import ExitStack

    with tile.TileContext(nc) as tc:
        with ExitStack() as ctx:
            statics = ctx.enter_context(tc.tile_pool(name="statics", bufs=1))
            pers = ctx.enter_context(tc.tile_pool(name="pers", bufs=1))
            sb2 = ctx.enter_context(tc.tile_pool(name="sb2", bufs=2))
            big = ctx.enter_context(tc.tile_pool(name="big", bufs=2))
            zp = ctx.enter_context(tc.tile_pool(name="zp", bufs=2, space="PSUM"))
            tp = ctx.enter_context(tc.tile_pool(name="tp", bufs=2, space="PSUM"))
            sp = ctx.enter_context(tc.tile_pool(name="sp", bufs=2, space="PSUM"))
            dsc = ctx.enter_context(tc.tile_pool(name="dsc", bufs=2,
                                                 space="DRAM"))
            # ---- load constants into SBUF once ----
            def ld(apD, dt=None):
                t = statics.tile(list(apD.shape), dt or apD.dtype,
                                 tag=f"c_{apD.name}")
                nc.sync.dma_start(out=t[:], in_=apD)
                return t

            embbd = ld(embbdD)
            I128 = ld(I128D)
            b1nr = ld(b1nrD)
            fb3r = ld(fb3rD)
            onesr = ld(onesrD)
            embb = ld(embbD)
            Abd = ld(AbdD)
            Bbd = ld(BbdD)
            b1c = ld(b1cD)
            Tbig2 = ld(Tbig2D)
            maskbd = ld(maskbdD)
            W2C1bd = ld(W2C1bdD)
            b2c1c = ld(b2c1cD)
            c2sc = ld(c2scD)
            W2bd = ld(W2bdD)
            b2x64 = ld(b2x64D)
            W1nh2 = ld(W1nh2D)
            W1nm2 = ld(W1nm2D)
            W2nbd = ld(W2nbdD)
            b2nc = ld(b2ncD)
            Wf3 = ld(Wf3D)

            # persistent per-parity combo tiles
            cA_lhs = []  # [66, 128]: rows 0:64 HAstack (per pair), 64:66 w1r
            cA_rhs = []  # [66, 4096]: rows 0:64 Tbig static, 64:66 d2 flat
            wT = []      # [128, 128] blockdiag w target, zeroed once
            for par in range(2):
                lt = pers.tile([N + 2, 2 * H], F32, tag=f"cAl{par}")
                nc.sync.dma_start(out=lt[N:N + 2, :], in_=w1rmD)
                rt = pers.tile([N + 2, NSQ], F32, tag=f"cAr{par}")
                nc.sync.dma_start(out=rt[0:N, :], in_=TbigD)
                wt = pers.tile([2 * H, 2 * H], F32, tag=f"wT{par}")
                nc.vector.memset(wt[:], 0.0)
                cA_lhs.append(lt)
                cA_rhs.append(rt)
                wT.append(wt)

            for p in range(PAIRS):
                par = p % 2
                # ---- per-pair loads ----
                x_pair = sb2.tile([2 * N, D], F32)
                nc.gpsimd.dma_start(
                    out=x_pair[:],
                    in_=xD[2 * p:2 * p + 2, :, :].rearrange(
                        "g i d -> (g i) d"),
                )
                # x^T via PE transpose + DRAM bounce into padded per-graph
                # layout (graph 0 rows 0:6, graph 1 rows 32:38 -- PE-legal
                # partition bases)
                pxT = sp.tile([D, 2 * N], F32, tag="sp")
                nc.tensor.transpose(pxT[:], x_pair[:], I128[:])
                xT6 = sb2.tile([D, 2 * N], F32)
                nc.vector.tensor_copy(out=xT6[:], in_=pxT[:])
                xts = dsc.tile([D, 2 * N], F32, tag="xts")
                nc.gpsimd.dma_start(out=xts[:], in_=xT6[:])
                xT12p = sb2.tile([38, N], F32)
                nc.vector.memset(xT12p[:], 0.0)
                for gg in range(2):
                    nc.gpsimd.dma_start(out=xT12p[32 * gg:32 * gg + D, :],
                                        in_=xts[:, gg * N:(gg + 1) * N])

                # ---- embedding (transposed pair layout) ----
                ph = sp.tile([2 * H, N], F32, tag="sp")
                nc.tensor.matmul(ph[:], _mm(embbd[:]), _mm(xT12p[:]),
                                 start=True, stop=True)
                hT2 = sb2.tile([2 * H, N], F32)
                nc.vector.tensor_scalar_add(hT2[:], ph[:], embb[:, 0:1])

                # ---- HA/HB stacks ----
                pHA = sp.tile([N, 2 * H], F32, tag="sp")
                nc.tensor.matmul(pHA[:], _mm(hT2[:]), _mm(Abd[:]),
                                 start=True, stop=True)
                nc.vector.tensor_copy(out=cA_lhs[par][0:N, :], in_=pHA[:])
                pHB = sp.tile([N, 2 * H], F32, tag="sp")
                nc.tensor.matmul(pHB[:], _mm(hT2[:]), _mm(Bbd[:]),
                                 start=True, stop=True)
                HBst = sb2.tile([N, 2 * H], F32)
                nc.scalar.copy(out=HBst[:], in_=pHB[:])

                # ---- pairwise distances d2 ----
                pm2c = sb2.tile([38, N], F32)
                nc.vector.tensor_scalar_mul(pm2c[:], xT12p[:], -2.0)
                posTbd = sb2.tile([38, 2 * N], F32)
                nc.vector.memset(posTbd[:], 0.0)
                for gg in range(2):
                    nc.vector.tensor_copy(
                        out=posTbd[32 * gg:32 * gg + 3,
                                   gg * N:(gg + 1) * N],
                        in_=xT12p[32 * gg:32 * gg + 3, :])
                n2c = sb2.tile([2 * N, 1], F32)
                sq3 = sb2.tile([2 * N, 3], F32)
                nc.vector.scalar_tensor_tensor(
                    out=sq3[:], in0=x_pair[:, 0:3], scalar=1.0,
                    in1=x_pair[:, 0:3], op0=OP.mult, op1=OP.mult,
                    accum_out=n2c[:, 0:1])
                n2s = dsc.tile([2 * N, 1], F32, tag="n2s")
                nc.gpsimd.dma_start(out=n2s[:], in_=n2c[:])
                n2r = sb2.tile([2, N], F32)
                nc.gpsimd.dma_start(
                    out=n2r[:],
                    in_=n2s[:].rearrange("(g j) o -> g (j o)", g=2))
                pg = sp.tile([2 * H, N], F32, tag="sp")
                nc.tensor.matmul(pg[:], _mm(posTbd[:]), _mm(pm2c[:]),
                                 start=True, stop=False)
                nc.tensor.matmul(pg[:], _mm(maskbd[:]), _mm(n2r[:]),
                                 start=False, stop=True)
                d2 = sb2.tile([2 * N, N], F32)
                nc.vector.tensor_scalar_add(d2[:], pg[:], n2c[:, 0:1])
                # flatten [(g j), i] -> combo rows [g, (j i)] via DRAM
                # bounce; valid because d2 is symmetric
                d2s = dsc.tile([2 * N, N], F32, tag="d2s")
                nc.gpsimd.dma_start(out=d2s[:], in_=d2[:])
                nc.gpsimd.dma_start(
                    out=cA_rhs[par][N:N + 2, :],
                    in_=d2s[:].rearrange("(g j) i -> g (j i)", g=2))

                # ---- big phase: z1 -> r1 -> t -> u -> w ----
                r1 = big.tile([2 * H, NSQ], dt_r1, tag="r1")
                u = big.tile([2 * H, NSQ], dt_r1, tag="u")
                pw = sp.tile([2 * NCH, CH], F32, tag="sp")
                for cb in range(NCH):
                    pz = zp.tile([2 * H, CH], F32, tag="z")
                    nc.tensor.matmul(
                        pz[:], _mm(cA_lhs[par][:]),
                        _mm(cA_rhs[par][:, cb * CH:(cb + 1) * CH]),
                        start=True, stop=False)
                    nc.tensor.matmul(
                        pz[:], _mm(HBst[:]),
                        _mm(Tbig2[:, cb * CH:(cb + 1) * CH]),
                        start=False, stop=True)
                    r1_sl = r1[:, cb * CH:(cb + 1) * CH]
                    if cb % 2 == 0:
                        nc.scalar.activation(out=r1_sl, in_=pz[:], func=AF.Relu,
                                             bias=b1c[:, 0:1], scale=1.0)
                    else:
                        nc.vector.tensor_scalar(
                            out=r1_sl, in0=pz[:], scalar1=b1c[:, 0:1],
                            scalar2=0.0, op0=OP.add, op1=OP.max)
                    pt = tp.tile([2 * H, CH], F32, tag="t")
                    nc.tensor.matmul(pt[:], W2C1bd[:].bitcast(_mmdt(R1DT)),
                                     _mm(r1_sl), start=True, stop=True)
                    u_sl = u[:, cb * CH:(cb + 1) * CH]
                    if cb % 2 == 1:
                        nc.scalar.activation(out=u_sl, in_=pt[:], func=AF.Relu,
                                             bias=b2c1c[:, 0:1], scale=1.0)
                    else:
                        nc.vector.tensor_scalar(
                            out=u_sl, in0=pt[:], scalar1=b2c1c[:, 0:1],
                            scalar2=0.0, op0=OP.add, op1=OP.max)
                    nc.tensor.matmul(pw[:],
                                     c2sc[:, 16 * cb:16 * (cb + 1)]
                                     .bitcast(_mmdt(R1DT)), _mm(u_sl),
                                     start=(cb == 0), stop=(cb == NCH - 1),
                                     skip_group_check=True)

                # ---- w -> blockdiag via SBUF + DRAM bounce ----
                w16 = sb2.tile([2 * NCH, CH], F32)
                nc.scalar.copy(out=w16[:], in_=pw[:])
                ws = dsc.tile([2 * NCH, CH], F32, tag="ws")
                nc.gpsimd.dma_start(out=ws[:], in_=w16[:])
                for gg in range(2):
                    # DRAM-side gather: row 2cb+gg, chunk-local (jl, i)
                    nc.gpsimd.dma_start(
                        out=wT[par][gg * N:(gg + 1) * N,
                                    gg * N:(gg + 1) * N],
                        in_=ws[:].rearrange("(cb g) (jl i) -> g cb jl i",
                                            g=2, jl=JB)[gg],
                    )

                # ---- R = sum_j r1 (log-tree fold over outer j) ----
                tr = big.tile([2 * H, NSQ // 2], dt_r1, tag="tr")
                nc.vector.tensor_add(tr[:], r1[:, 0:NSQ // 2],
                                     r1[:, NSQ // 2:NSQ])
                wdt = NSQ // 4
                while wdt >= N:
                    nc.vector.tensor_add(tr[:, 0:wdt], tr[:, 0:wdt],
                                         tr[:, wdt:2 * wdt])
                    wdt //= 2
                # ---- m = R @ W2 + 64*b2 (T layout) ----
                pm = sp.tile([2 * H, N], F32, tag="sp")
                nc.tensor.matmul(pm[:], W2bd[:].bitcast(_mmdt(R1DT)),
                                 _mm(tr[:, 0:N]), start=True, stop=True)
                mTs = sb2.tile([2 * H, N], F32)
                nc.vector.tensor_scalar_add(mTs[:], pm[:], b2x64[:, 0:1])

                # ---- node MLP ----
                # init-mm seeds the full bank with the bias so the per-graph
                # matmuls can all accumulate (single start per bank region)
                pq = sp.tile([2 * H, N], F32, tag="sp")
                nc.tensor.matmul(pq[:], _mm(b1nr[:]), _mm(onesr[:, 0:N]),
                                 start=True, stop=False)
                for gg in range(2):
                    o = pq[gg * H:(gg + 1) * H, :]
                    sl = slice(gg * H, (gg + 1) * H)
                    nc.tensor.matmul(o, _mm(W1nh2[sl, :]), _mm(hT2[sl, :]),
                                     start=False, stop=False)
                    nc.tensor.matmul(o, _mm(W1nm2[sl, :]), _mm(mTs[sl, :]),
                                     start=False, stop=(gg == 1))
                qT = sb2.tile([2 * H, N], F32)
                nc.scalar.activation(out=qT[:], in_=pq[:], func=AF.Relu,
                                     bias=0.0, scale=1.0)
                pn2 = sp.tile([2 * H, N], F32, tag="sp")
                nc.tensor.matmul(pn2[:], _mm(W2nbd[:]), _mm(qT[:]),
                                 start=True, stop=True)
                hp = sb2.tile([2 * H, N], F32)
                nc.vector.scalar_tensor_tensor(
                    out=hp[:], in0=pn2[:], scalar=b2nc[:, 0:1], in1=hT2[:],
                    op0=OP.add, op1=OP.add)

                # ---- coordinate aggregation ----
                pa = sb2.tile([2 * N, 4], F32)
                nc.vector.tensor_copy(out=pa[:, 0:3], in_=x_pair[:, 0:3])
                nc.vector.memset(pa[:, 3:4], 1.0)
                pswp = sp.tile([2 * N, 4], F32, tag="sp")
                nc.tensor.matmul(pswp[:], _mm(wT[par][:]), _mm(pa[:]),
                                 start=True, stop=True)

                # ---- velocity head ----
                pv = sp.tile([2 * N, 3], F32, tag="sp")
                nc.tensor.matmul(pv[:], _mm(onesr[:]), _mm(fb3r[:]),
                                 start=True, stop=False)
                for gg in range(2):
                    sl = slice(gg * H, (gg + 1) * H)
                    nc.tensor.matmul(pv[gg * N:(gg + 1) * N, :],
                                     _mm(hp[sl, :]), _mm(Wf3[sl, :]),
                                     start=False, stop=(gg == 1))

                # ---- assemble output ----
                op_t = sb2.tile([2 * N, D], F32)
                tmp3 = sb2.tile([2 * N, 3], F32)
                nc.vector.scalar_tensor_tensor(
                    out=tmp3[:], in0=x_pair[:, 0:3], scalar=pswp[:, 3:4],
                    in1=pswp[:, 0:3], op0=OP.mult, op1=OP.subtract)
                nc.vector.scalar_tensor_tensor(
                    out=op_t[:, 0:3], in0=tmp3[:], scalar=1.0 / N,
                    in1=x_pair[:, 0:3], op0=OP.mult, op1=OP.add)
                nc.vector.tensor_add(op_t[:, 3:6], pv[:], x_pair[:, 3:6])
                nc.gpsimd.dma_start(
                    out=outD[2 * p:2 * p + 2, :, :].rearrange(
                        "g i d -> (g i) d"),
                    in_=op_t[:])

    nc.compile()
    return nc


def make_consts(emb_w, emb_b, edge_w1, edge_b1, edge_w2, edge_b2,
                node_w1, node_b1, node_w2, node_b2,
                coord_w1, coord_b1, coord_w2, final_w, final_b):
    f = np.float32
    E = np.asarray(emb_w, f)
    A = np.asarray(edge_w1[0:H], f)
    Bm = np.asarray(edge_w1[H:2 * H], f)
    w1r = np.asarray(edge_w1[2 * H], f)
    W2 = np.asarray(edge_w2, f)
    C1 = np.asarray(coord_w1, f)
    W2C1 = (W2 @ C1).astype(f)
    b2c1 = (np.asarray(edge_b2, f) @ C1 + np.asarray(coord_b1, f)).astype(f)
    c2 = np.asarray(coord_w2, f)[:, 0]

    def bd(M):
        out = np.zeros((2 * M.shape[0], 2 * M.shape[1]), f)
        out[:M.shape[0], :M.shape[1]] = M
        out[M.shape[0]:, M.shape[1]:] = M
        return out

    def dup(v):
        return np.tile(np.asarray(v, f), 2)[:, None].astype(f)

    w1rm = np.zeros((2, 2 * H), f)
    w1rm[0, 0:H] = w1r
    w1rm[1, H:2 * H] = w1r
    # c2sc: for chunk cb, lhsT = c2sc[:, 16cb:16cb+16]; column m of that
    # slice carries c2 masked to graph gg iff m == 2*cb+gg, so the 8
    # accumulating matmuls scatter chunk cb's w into psum rows 2cb:2cb+2.
    c2sc = np.zeros((2 * H, 16 * NCH), f)
    for cb in range(NCH):
        for gg in range(2):
            c2sc[gg * H:(gg + 1) * H, 16 * cb + 2 * cb + gg] = c2
    maskbd = np.zeros((2, 2 * H), f)
    maskbd[0, 0:N] = 1.0
    maskbd[1, N:2 * N] = 1.0
    r1dt = np.dtype("bfloat16") if R1_BF16 else f
    W1 = np.asarray(node_w1, f)
    embbd38 = np.zeros((38, 2 * H), f)
    embbd38[0:D, 0:H] = E
    embbd38[32:32 + D, H:2 * H] = E
    consts = {
        "embbd": embbd38,
        "I128": np.eye(2 * H, dtype=f),
        "b1nr": np.tile(np.asarray(node_b1, f), 2)[None, :],
        "fb3r": np.asarray(final_b, f)[None, 3:6],
        "onesr": np.ones((1, 2 * H), f),
        "embb": dup(emb_b),
        "Abd": bd(A),
        "Bbd": bd(Bm),
        "w1rm": w1rm,
        "b1c": dup(edge_b1),
        "Tbig": np.tile(np.eye(N, dtype=f), (1, N)),
        "Tbig2": np.kron(np.eye(N, dtype=f), np.ones((1, N), f)),
        "maskbd": maskbd,
        "W2C1bd": bd(W2C1).astype(r1dt),
        "b2c1c": dup(b2c1),
        "c2sc": c2sc.astype(r1dt),
        "W2bd": bd(W2).astype(r1dt),
        "b2x64": dup(np.asarray(edge_b2, f) * N),
        "W1nh2": np.concatenate([W1[0:H], W1[0:H]], 0),
        "W1nm2": np.concatenate([W1[H:2 * H], W1[H:2 * H]], 0),
        "W2nbd": bd(np.asarray(node_w2, f)),
        "b2nc": dup(node_b2),
        "Wf3": np.tile(np.asarray(final_w, f)[:, 3:6], (2, 1)),
    }
    return consts


_CACHE = {}


def _get_program():
    if "nc" not in _CACHE:
        _CACHE["nc"] = build_program()
    return _CACHE["nc"]


def _build_runner():
    """Build a cached jit(shard_map(bass_exec)) runner.

    The per-call wall time in this environment is dominated by the host
    <-> device tunnel: each host numpy argument costs ~2-7 ms to ship
    per call and the dispatch itself has a large fixed latency.  So:
      - the jitted callable is built ONCE and reused (no per-call
        retrace/relower),
      - all weight-derived constant tensors are uploaded once and kept
        device-resident (refreshed only if the weight values change),
      - the ExternalOutput seed buffers are device-resident and NOT
        donated (the kernel fully overwrites the output, so their
        content never matters and they can be reused every call),
      - only `x` (0.4 MB total) is shipped per call.
    """
    import jax
    from jax.sharding import Mesh, PartitionSpec, NamedSharding
    try:
        from jax.experimental.shard_map import shard_map

        def _shard_map(f, **kw):
            return shard_map(f, check_rep=False, **kw)
    except ImportError:
        from jax import shard_map

        def _shard_map(f, **kw):
            return shard_map(f, check_vma=False, **kw)
    from concourse import bass2jax

    nc = _get_program()
    bass2jax.install_neuronx_cc_hook()

    partition_name = (nc.partition_id_tensor.name
                      if nc.partition_id_tensor else None)
    in_names = []
    out_names = []
    out_avals = []
    for alloc in nc.m.functions[0].allocations:
        if not isinstance(alloc, mybir.MemoryLocationSet):
            continue
        name = alloc.memorylocations[0].name
        if alloc.kind == "ExternalInput":
            if name != partition_name:
                in_names.append(name)
        elif alloc.kind == "ExternalOutput":
            out_names.append(name)
            shape = tuple(alloc.tensor_shape)
            dtype = mybir.dt.np(alloc.dtype)
            out_avals.append(jax.core.ShapedArray(shape, dtype))
    param_names = list(in_names)
    n_params = len(param_names)
    bind_names = list(in_names) + list(out_names)
    if partition_name is not None:
        bind_names.append(partition_name)

    def _body(*args):
        operands = list(args)
        if partition_name is not None:
            operands.append(bass2jax.partition_id_tensor())
        outs = bass2jax._bass_exec_p.bind(
            *operands,
            out_avals=tuple(out_avals),
            in_names=tuple(bind_names),
            out_names=tuple(out_names),
            lowering_input_output_aliases=(),
            sim_require_finite=True,
            sim_require_nnan=True,
            nc=nc,
        )
        return tuple(outs)

    devices = jax.devices()[:NCORES]
    assert len(devices) == NCORES
    mesh = Mesh(np.asarray(devices), ("core",))
    P = PartitionSpec
    nargs = n_params + len(out_names)
    sharded = jax.jit(
        _shard_map(_body, mesh=mesh, in_specs=(P("core"),) * nargs,
                   out_specs=(P("core"),) * len(out_names)),
        keep_unused=True,
    )
    sh = NamedSharding(mesh, P("core"))

    def put(arr_per_core):
        g = np.concatenate([np.asarray(arr_per_core)] * NCORES, axis=0)
        return jax.device_put(g, sh)

    out_seeds = [
        jax.device_put(
            np.zeros((NCORES * av.shape[0], *av.shape[1:]), av.dtype), sh)
        for av in out_avals
    ]
    return {
        "sharded": sharded,
        "param_names": param_names,
        "out_seeds": out_seeds,
        "put": put,
        "sh": sh,
        "jax": jax,
    }


def _np_spot_reference(xg, emb_w, emb_b, edge_w1, edge_b1, edge_w2, edge_b2,
                       node_w1, node_b1, node_w2, node_b2,
                       coord_w1, coord_b1, coord_w2, final_w, final_b):
    """Exact numpy mirror of the reference model for a batch of graphs
    xg [g, N, D] -> [g, N, D].  Used only for first-call validation."""
    f = np.float32
    xg = np.asarray(xg, f)
    relu = lambda a: np.maximum(a, 0.0)
    pos, vel = xg[..., :3], xg[..., 3:]
    h = xg @ np.asarray(emb_w, f) + np.asarray(emb_b, f)
    rel = pos[:, :, None, :] - pos[:, None, :, :]
    d2 = np.sum(rel * rel, axis=-1, keepdims=True)
    g, n, hd = h.shape
    h_i = np.broadcast_to(h[:, :, None, :], (g, n, n, hd))
    h_j = np.broadcast_to(h[:, None, :, :], (g, n, n, hd))
    e_in = np.concatenate([h_i, h_j, d2], axis=-1)
    m_ij = relu(e_in @ np.asarray(edge_w1, f) + np.asarray(edge_b1, f)) \
        @ np.asarray(edge_w2, f) + np.asarray(edge_b2, f)
    m_i = m_ij.sum(axis=2)
    w = relu(m_ij @ np.asarray(coord_w1, f) + np.asarray(coord_b1, f)) \
        @ np.asarray(coord_w2, f)
    pos_out = pos + (rel * w).sum(axis=2) / n
    n_in = np.concatenate([h, m_i], axis=-1)
    h = h + relu(n_in @ np.asarray(node_w1, f) + np.asarray(node_b1, f)) \
        @ np.asarray(node_w2, f) + np.asarray(node_b2, f)
    vel_out = vel + (h @ np.asarray(final_w, f) + np.asarray(final_b, f))[..., 3:]
    return np.concatenate([pos_out, vel_out], axis=-1)


def _quick_ok(out):
    # strided sample (~50 elems/graph) — catches the observed global-garbage
    # corruption mode at ~10x lower cost than a full scan
    s = out.reshape(-1)[::31]
    return bool(np.isfinite(s).all()) and float(np.abs(s).max()) < 1e6


def _spot_ok(out, xf, weights):
    """Validate one graph per core against the numpy reference."""
    idx = np.arange(0, G, GPC)
    exp = _np_spot_reference(xf[idx], *weights)
    got = out.reshape(G, N, D)[idx]
    scale = max(float(np.abs(exp).max()), 1e-6)
    return float(np.abs(got - exp).max()) / scale < 1e-3


def _fast_call(consts_fn, wkey, xf):
    r = _CACHE.get("runner")
    if r is None:
        r = _build_runner()
        _CACHE["runner"] = r
    if _CACHE.get("consts_key") != wkey:
        consts = consts_fn()
        cdev = {k: r["put"](v) for k, v in consts.items()}
        for a in cdev.values():
            a.block_until_ready()
        _CACHE["consts_dev"] = cdev
        _CACHE["consts_key"] = wkey
    cdev = _CACHE["consts_dev"]
    args = []
    for name in r["param_names"]:
        if name == "x":
            args.append(xf)  # global [G, N, D]: concat of per-core slices
        else:
            args.append(cdev[name])
    args.extend(r["out_seeds"])
    outs = r["sharded"](*args)
    return np.asarray(outs[0])


def kernel(x, emb_w, emb_b, edge_w1, edge_b1, edge_w2, edge_b2,
           node_w1, node_b1, node_w2, node_b2,
           coord_w1, coord_b1, coord_w2, final_w, final_b,
           _return_bass_results=False, _trace=False):
    import hashlib
    weights = (emb_w, emb_b, edge_w1, edge_b1, edge_w2, edge_b2,
               node_w1, node_b1, node_w2, node_b2,
               coord_w1, coord_b1, coord_w2, final_w, final_b)
    h = hashlib.blake2b(digest_size=16)
    for w in weights:
        h.update(np.ascontiguousarray(np.asarray(w, np.float32)).tobytes())
    wkey = h.digest()
    consts_fn = lambda: make_consts(*weights)
    xf = np.ascontiguousarray(np.asarray(x, np.float32).reshape(G, N, D))
    out = None
    if not _CACHE.get("fast_dead"):
        try:
            out = _fast_call(consts_fn, wkey, xf)
            if not _quick_ok(out):
                out = None
            if out is not None and _CACHE.get("validated_key") != wkey:
                if _spot_ok(out, xf, weights):
                    _CACHE["validated_key"] = wkey
                else:
                    out = None
            if out is None:
                # one retry with a fresh constant upload
                _CACHE.pop("consts_key", None)
                out = _fast_call(consts_fn, wkey, xf)
                if not _quick_ok(out) or not _spot_ok(out, xf, weights):
                    out = None
                    _CACHE["fast_dead"] = True
                else:
                    _CACHE["validated_key"] = wkey
        except Exception:
            out = None
            _CACHE["fast_dead"] = True
    if out is None:
        # fallback: the original (slow but known-good) spmd path
        consts = consts_fn()
        nc = _get_program()
        in_maps = []
        for c in range(NCORES):
            m = dict(consts)
            m["x"] = np.ascontiguousarray(xf[c * GPC:(c + 1) * GPC])
            in_maps.append(m)
        res = run_bass_kernel_spmd(nc, in_maps,
                                   core_ids=list(range(NCORES)))
        out = np.concatenate(
            [res.results[c]["out"] for c in range(NCORES)], 0)
    out = np.asarray(out.reshape(B, S, N, D), np.float32)  # no-copy when f32
    if _return_bass_results:
        from types import SimpleNamespace
        return out, SimpleNamespace(exec_time_ns=None, results=None)
    return out

